# revision 4
# baseline (speedup 1.0000x reference)
"""Trainium2 Bass kernel for nn_Attention_39676907884025.

out[b, q, :] = (1/SK) * sum_k value[b, k, :] for every q: q_param (1x1) is
broadcast over query and key, the score matrix is constant along the softmax
axis, and softmax of a constant row is exactly uniform. Only `value` touches
the device; batch B=16 is data-parallel over 8 cores (2 per core).

Raw bacc, hand-scheduled. HW-measured 27.3-29.3us vs 27.6-32.2us for the
tile-framework baseline across device thermal states (paired same-state
runs: 4+us faster). Design notes:
  - DMA only on the two HWDGE rings (SWDGE/gpsimd queues measure ~40%
    slower per descriptor and steal SDMA slots): 4 quarter-chunk loads +
    4 quarter stores per batch (256 KB, 2 KB descriptors), one dedicated
    completion semaphore per load chunk (shared counting sems are racy:
    the 16 SDMA engine slots increment independently).
  - DVE tree-reduces chunks as they land, dependent ops kept >= 2 apart
    (interleaved order + 8-col spacer copies) so bacc inserts none of its
    ~450 ns same-engine pipeline drains; final fold casts the accumulator
    to bf16 for free (rel err 1.7e-3 << 2e-2 tolerance).
  - PE: one single-pass bf16 matmul per batch (constant 1/SK stationary)
    reduces across partitions AND broadcasts the mean into fp32 PSUM.
  - ACT alone widens PSUM -> (128, 512) via four independent PSUM reads
    (no dependent pair -> no ACT drain) and is the only engine waiting on
    the PE semaphore: DVE/Pool waiting on PE event-accel semaphores
    deadlocks the device, as does ending Pool with in-flight SWDGE DMAs.


out[b, q, :] = (1/SK) * sum_k value[b, k, :]  for every q (softmax of a
constant score matrix is exactly uniform).

Raw bacc. Per core (2 batches), proven-safe sync patterns only:
  - 4 load chunks per batch (256 KB, 2 KB descriptors) over 3 queues,
    dedicated sem per chunk.
  - DVE: interleaved tree schedule with hazard distance >= 2 between
    dependent ops (plus tiny dummy spacers), so bacc inserts no ~450 ns
    pipeline drains; final fold writes acc in bf16 (output cast is free,
    |mean| error ~0.3% << 2e-2 budget).
  - PE: one 1-pass bf16 matmul per batch (constant 1/SK stationary) ->
    fp32 PSUM mean tile broadcast to all 128 rows.
  - ACT alone widens PSUM -> (128, 512) SBUF (s_mm has a single waiter;
    DVE/PE never wait on event-accel sems from PE - that pattern hangs
    the hardware, see kernels 3-6/9).
  - stores: 4 per batch (256 KB, 2 KB descriptors): sync/act one each,
    gpsimd two.
"""

import sys

import numpy as np

if "/opt/trn_rl_repo" not in sys.path:
    sys.path.insert(0, "/opt/trn_rl_repo")

B, SQ, SK, D, DV = 16, 2048, 2048, 128, 128
N_CORES = 8
BPC = B // N_CORES  # batches per core
P = 128

LAST_RESULT = None  # BassKernelResults of the most recent run (for profiling)


def _build_nc():
    import concourse.bacc as bacc
    import concourse.mybir as mybir

    f32 = mybir.dt.float32
    bf16 = mybir.dt.bfloat16
    nc = bacc.Bacc("TRN2", target_bir_lowering=False)

    val = nc.dram_tensor("value", [BPC, SK, DV], f32, kind="ExternalInput")
    out = nc.dram_tensor("out", [BPC, SQ, DV], f32, kind="ExternalOutput")

    w = nc.alloc_sbuf_tensor("w_const", [P, P], bf16)
    xts = [nc.alloc_sbuf_tensor(f"xt{b}", [P, SK], f32) for b in range(BPC)]
    # level-1 tree scratch per chunk c: [256c, 256c+256)
    lv1 = [nc.alloc_sbuf_tensor(f"lv1_{b}", [P, 1024], f32) for b in range(BPC)]
    # per-chunk 128-col tree sums at [128c, 128c+128)
    qac = [nc.alloc_sbuf_tensor(f"qac_{b}", [P, 512], f32) for b in range(BPC)]
    # fold partials: F1 at [0:128], F2 at [128:256]
    pr = [nc.alloc_sbuf_tensor(f"pr_{b}", [P, 256], f32) for b in range(BPC)]
    acc = [nc.alloc_sbuf_tensor(f"acc_{b}", [P, P], bf16) for b in range(BPC)]
    wide = [nc.alloc_sbuf_tensor(f"wide{b}", [P, 512], f32) for b in range(BPC)]
    dum = nc.alloc_sbuf_tensor("dum", [P, 128], f32)
    pss = [nc.alloc_psum_tensor(f"ps{b}", [P, P], f32) for b in range(BPC)]

    s_ld = [
        [nc.alloc_semaphore(f"s_ld_{b}_{c}") for c in range(4)] for b in range(BPC)
    ]
    s_sp = nc.alloc_semaphore("s_sp")
    s_act = nc.alloc_semaphore("s_act")
    s_w = nc.alloc_semaphore("s_w")
    s_dve = nc.alloc_semaphore("s_dve")
    s_mm = nc.alloc_semaphore("s_mm")
    s_wide = nc.alloc_semaphore("s_wide")

    def xdst(b):
        return xts[b][:].rearrange("p (t d) -> p t d", d=DV)

    def xsrc(b):
        return val[b].rearrange("(p t) d -> p t d", p=P)

    def odst(b):
        return out[b].rearrange("(p t) d -> p t d", p=P)

    def wsrc(b):
        return wide[b][:].rearrange("p (t d) -> p t d", d=DV)

    def load(eng, b, c):
        return eng.dma_start(
            xdst(b)[:, 4 * c : 4 * c + 4, :], xsrc(b)[:, 4 * c : 4 * c + 4, :]
        ).then_inc(s_ld[b][c], 16)

    def store(eng, b, t0, sem):
        return eng.dma_start(
            odst(b)[:, t0 : t0 + 4, :], wsrc(b)
        ).then_inc(sem, 16)

    with nc.Block() as block:

        @block.sync
        def _(sync):
            load(sync, 0, 0)
            load(sync, 0, 2)
            load(sync, 1, 1)
            load(sync, 1, 3)
            sync.wait_ge(s_wide, 1)
            store(sync, 0, 0, s_sp)
            store(sync, 0, 8, s_sp)
            sync.wait_ge(s_wide, 2)
            store(sync, 1, 0, s_sp)
            store(sync, 1, 8, s_sp)
            sync.wait_ge(s_sp, 64)

        @block.scalar
        def _(scalar):
            load(scalar, 0, 1)
            load(scalar, 0, 3)
            load(scalar, 1, 0)
            load(scalar, 1, 2)
            for b in range(BPC):
                # widen: replicate psum mean tile 4x into wide[b]
                scalar.wait_ge(s_mm, b + 1)
                scalar.copy(wide[b][:, 0:P], pss[b][:])
                scalar.copy(wide[b][:, P : 2 * P], pss[b][:])
                scalar.copy(wide[b][:, 2 * P : 3 * P], pss[b][:])
                scalar.copy(wide[b][:, 3 * P : 4 * P], pss[b][:]).then_inc(
                    s_wide, 1
                )
                scalar.wait_ge(s_wide, b + 1)
                store(scalar, b, 4, s_act)
                store(scalar, b, 12, s_act)
            scalar.wait_ge(s_act, 64)

        @block.vector
        def _(vector):
            vector.memset(w[:], 1.0 / SK).then_inc(s_w, 1)

            def a1(b, c):
                # level-1: (128, 512) chunk -> 256 partial sums
                vector.wait_ge(s_ld[b][c], 16)
                lo = 512 * c
                vector.tensor_add(
                    lv1[b][:, 256 * c : 256 * c + 256],
                    xts[b][:, lo : lo + 256],
                    xts[b][:, lo + 256 : lo + 512],
                )

            def a2(b, c):
                # level-2: 256 -> 128 (chunk sum s_c)
                vector.tensor_add(
                    qac[b][:, 128 * c : 128 * c + 128],
                    lv1[b][:, 256 * c : 256 * c + 128],
                    lv1[b][:, 256 * c + 128 : 256 * c + 256],
                )

            def dummy():
                # spacer: keeps dependent ops >= 2 apart so no pipe drain
                vector.tensor_copy(dum[:, 0:8], qac[0][:, 0:8])

            def batch(b):
                # interleaved: every dependent pair has >= 2 ops between
                a1(b, 0)                                       # L1 c0
                a1(b, 1)                                       # L1 c1
                a2(b, 0)                                       # s0
                a1(b, 2)                                       # L1 c2
                a2(b, 1)                                       # s1
                a2(b, 2)                                       # s2
                vector.tensor_add(                             # F1 = s0+s1
                    pr[b][:, 0:128], qac[b][:, 0:128], qac[b][:, 128:256]
                )
                dummy()
                vector.tensor_add(                             # F2 = F1+s2
                    pr[b][:, 128:256], pr[b][:, 0:128], qac[b][:, 256:384]
                )
                a1(b, 3)                                       # L1 c3
                dummy()
                a2(b, 3)                                       # s3
                dummy()
                vector.tensor_add(                             # acc = F2+s3
                    acc[b][:], pr[b][:, 128:256], qac[b][:, 384:512]
                ).then_inc(s_dve, 1)

            batch(0)
            batch(1)

        @block.tensor
        def _(tensor):
            tensor.wait_ge(s_w, 1)
            for b in range(BPC):
                tensor.wait_ge(s_dve, b + 1)
                nc.tensor.matmul(
                    pss[b][:], w[:], acc[b][:], start=True, stop=True
                ).then_inc(s_mm, 1)

    nc.compile()
    return nc


def kernel(query=None, key=None, value=None, q_param=None, _trace=False):
    from concourse.bass_utils import run_bass_kernel_spmd

    global LAST_RESULT

    value = np.ascontiguousarray(np.asarray(value, dtype=np.float32))
    assert value.shape == (B, SK, DV), value.shape

    nc = _build_nc()
    shards = value.reshape(N_CORES, BPC, SK, DV)
    in_maps = [{"value": shards[i]} for i in range(N_CORES)]

    LAST_RESULT = run_bass_kernel_spmd(
        nc, in_maps, list(range(N_CORES)), trace=_trace
    )
    return np.concatenate(
        [LAST_RESULT.results[i]["out"] for i in range(N_CORES)], axis=0
    )
